# revision 5
# baseline (speedup 1.0000x reference)
"""Bass/Trainium2 kernel for a 2-layer LSTM (B=512, T=2048, I=3, H=64).

Raw-bass (no TileContext) fused-layer design with hand-rolled semaphores:
every engine instruction carries at most ONE semaphore wait (its critical
RAW dependency); all WAR hazards are covered transitively through the
chain structure, so no standalone EventSemaphore instructions are needed
in steady state and the real data wait rides on the instruction itself
(parking in the engine wait queue instead of blocking the sequencer).

Math and layout identical to the tile version (kernel2):
  - layers fused on partitions: L1 rows 0-63, L2 rows 64-127; L2 lags one
    step.  Gate PSUM colblocks f,i,g,o; K-stacked state matmuls on
    st=[2h1;2h2] (fp16); c2x=2c (fp32).
  - per tick/group: u=(ti+1)tg; w=(tf+1)c2x; c2x=0.5w+u; tc=tanh(0.5c2x);
    st=(to+1)tc.

Sync plan per tick t, group g (sems: sP=PE matmuls, sA=ACT, sD=DVE,
sM=DMA):
  x-MMs:      no wait (chunk-boundary first MM waits sM)
  state-MM0:  wait sD >= ht(t-1,g)      (MM1-3: none, in-order)
  gates ACT:  wait sP >= stateMM3(t,g)
  u:          wait sA >= gates(t,g)
  w:          none (in-order after u)
  c:          wait sD >= w(t,g)         (same-engine pipelined-write)
  tc ACT:     wait sD >= c(t,g)
  ht:         wait sA >= tc(t,g)
  chunk DMA:  wait sP >= last x-MM of the buffer's previous tenant
"""

import numpy as np

B, T, I, H = 512, 2048, 3, 64
NCORES = 8
BL = B // NCORES  # 64 batch per core
BGS = [16, 24, 24]  # batch-group sizes (independent chains per core)
G = len(BGS)
BOFF = [sum(BGS[:g]) for g in range(G)]
CH = 64  # timesteps per x-chunk DMA

_CACHE = {}

_GATES = ["f", "i", "g", "o"]  # colblock order
_ROWS = {"i": slice(0, H), "f": slice(H, 2 * H), "g": slice(2 * H, 3 * H),
         "o": slice(3 * H, 4 * H)}
_SG = {"i": 0.5, "f": 0.5, "g": 1.0, "o": 0.5}


def _prep_weights(W_ih0, W_hh0, b_ih0, b_hh0, W_ih1, W_hh1, b_ih1, b_hh1):
    """Pack host-side lhsT weights (fp16). See kernel2 docstring."""
    b0 = b_ih0 + b_hh0
    b1 = b_ih1 + b_hh1
    wst = np.zeros((128, 512), np.float32)
    wx = np.zeros((4, 512), np.float32)
    for cb, gate in enumerate(_GATES):
        r = _ROWS[gate]
        sg = _SG[gate]
        c0 = cb * 128
        wst[0:64, c0:c0 + 64] = (W_hh0[r] * sg * 0.5).T
        wx[0:3, c0:c0 + 64] = (W_ih0[r] * sg).T
        wx[3, c0:c0 + 64] = b0[r] * sg
        wst[0:64, c0 + 64:c0 + 128] = (W_ih1[r] * sg * 0.5).T
        wst[64:128, c0 + 64:c0 + 128] = (W_hh1[r] * sg * 0.5).T
        wx[3, c0 + 64:c0 + 128] = b1[r] * sg
    return wst.astype(np.float16), wx.astype(np.float16)


def build_program(t_steps=T, bl=BL):
    from concourse import bacc, mybir

    f32 = mybir.dt.float32
    f16 = mybir.dt.float16
    Tanh = mybir.ActivationFunctionType.Tanh
    ADD = mybir.AluOpType.add
    MULT = mybir.AluOpType.mult

    nc = bacc.Bacc("TRN2", target_bir_lowering=False, debug=False)

    # xt carries a 512-col wx prefix so one DMA lands wx + chunk 0
    xt_d = nc.dram_tensor("xt", [4, 512 + t_steps * bl], f16,
                          kind="ExternalInput")
    wst_d = nc.dram_tensor("wst", [128, 512], f16, kind="ExternalInput")
    out_d = nc.dram_tensor("out", [64, bl], f16, kind="ExternalOutput")

    n_chunks = (t_steps + CH - 1) // CH
    PSR = 2  # psum ring depth per group

    with nc.cleanup_on_exit():
        sP = nc.alloc_semaphore("sP")
        sA = nc.alloc_semaphore("sA")
        sD = nc.alloc_semaphore("sD")
        sM = nc.alloc_semaphore("sM")
        cnt = {"P": 0, "A": 0, "D": 0, "M": 0}

        def inc(inst, which, sem, by=1):
            inst.then_inc(sem, by)
            cnt[which] += by
            return cnt[which]

        # --- sbuf/psum tensors ---
        wst = nc.alloc_sbuf_tensor("wst_s", [128, 512], f16)
        xch = [nc.alloc_sbuf_tensor(f"xch{r}", [4, 512 * (r == 0) + CH * bl],
                                    f16) for r in range(2)]
        wx = xch[0]  # cols 0:512 of xch0, loaded by the first chunk DMA
        # t1e layout: colblocks [f | i | c2x | g | o], each bg wide.  The
        # gates ACT writes [f,i] and [g,o] via one sub-strided AP; c2x (fp16)
        # is owned by the cell update.  [f,i] and [c2x,g] are contiguous
        # operand pairs for the paired cell STT.
        # one backing tensor for all group states: the final hidden rows
        # DMA out directly (f16; host applies the 0.5 and f32 cast)
        st_all = nc.alloc_sbuf_tensor("st_all", [128, bl], f16)
        sts = [st_all.ap()[:, BOFF[g]:BOFF[g] + BGS[g]] for g in range(G)]
        t1s = [nc.alloc_sbuf_tensor(f"t1{g}", [128, 5 * BGS[g]], f16)
               for g in range(G)]
        uws = [nc.alloc_sbuf_tensor(f"uw{g}", [128, 2 * BGS[g]], f16)
               for g in range(G)]
        tcs = [nc.alloc_sbuf_tensor(f"tc{g}", [128, BGS[g]], f16)
               for g in range(G)]

        import bass_rust as _br

        def gates_out_ap(g):
            """[128, 2, 2bg] view of t1e hitting cols [f,i] then [g,o]."""
            bg = BGS[g]
            a = t1s[g].ap().copy()
            a.ap = _br.VecI64Pair([[5 * bg, 128], [3 * bg, 2], [1, 2 * bg]])
            return a
        pss = [[nc.alloc_psum_tensor(f"ps{g}_{r}", [128, 4 * BGS[g]], f32)
                for r in range(PSR)] for g in range(G)]

        # --- preload first chunk + weights (chunk0 first: HWDGE issues
        # serially, so the tick-0 critical path clears ~780ns sooner; the
        # first x-MM's single sM wait at the post-wx count transitively
        # covers chunk0/wst/wx since DMA sem counts are emission-ordered) ---
        chunk_dma_count = [None] * n_chunks  # sM count when chunk ci loaded
        chunk_last_reader = [0] * 2  # sP count of last x-MM using buffer r

        def fetch_chunk(ci, wait_pe=None):
            # chunk ci source starts after the 512-col wx prefix; chunk 0's
            # DMA spans the prefix too, landing wx and chunk 0 together.
            # Even chunks reload xch0 cols 512: only, keeping wx intact.
            lo = 512 + ci * CH * bl
            hi = 512 + min((ci + 1) * CH, t_steps) * bl
            pre = 512 if ci == 0 else 0
            base = 0 if ci == 0 else 512 * (ci % 2 == 0)
            d = nc.sync.dma_start(
                xch[ci % 2].ap()[0:4, base:base + pre + hi - lo],
                xt_d.ap()[:, lo - pre:hi])
            if wait_pe:
                d._wait_ge(sP, wait_pe)
            chunk_dma_count[ci] = inc(d, "M", sM, 16)

        fetch_chunk(0)
        inc(nc.sync.dma_start(wst.ap()[:, :], wst_d.ap()[:, :]), "M", sM, 16)
        wst_cnt = cnt["M"]  # chunk0+wx, then wst, landed at this count
        if n_chunks > 1:
            fetch_chunk(1)

        # zero the c2x blocks (st needs no zeroing: tick 0 skips the state
        # matmuls since st=0 contributes nothing, and ht(0) overwrites st)
        for g in range(G):
            bg = BGS[g]
            inc(nc.vector.memset(t1s[g].ap()[:, 2 * bg:3 * bg], 0.0), "D", sD)

        ht_cnt = [cnt["D"]] * G  # sD count after ht(t-1, g)
        first_mm = True
        wst_wait = True  # tick-1 x-MM carries the wst DMA wait

        def xslice(t, g):
            ci, off = divmod(t, CH)
            base = 512 * (ci % 2 == 0) + off * bl + BOFF[g]
            return xch[ci % 2].ap()[0:4, base:base + BGS[g]]

        for t in range(t_steps + 1):
            ci = t // CH
            if t % CH == 1 and ci + 1 < n_chunks:
                # prefetch next chunk into the buffer last used by ci-1
                fetch_chunk(ci + 1,
                            wait_pe=chunk_last_reader[(ci + 1) % 2] or None)

            mm_cnt = [0] * G
            g_cnt = [0] * G
            c_cnt = [0] * G
            t_cnt = [0] * G
            for g in range(G):
                bg = BGS[g]
                ps = pss[g][t % PSR].ap()
                xr = xslice(min(t, t_steps - 1), g)
                st = sts[g]
                # x matmuls (start accumulation; tick 0 has no state
                # matmuls -- st is zero -- so cb3 closes the psum group)
                for cb in range(4):
                    mm = nc.tensor.matmul(ps[:, cb * bg:(cb + 1) * bg],
                                          wx.ap()[0:4,
                                                  cb * 128:(cb + 1) * 128],
                                          xr, start=cb == 0,
                                          stop=(t == 0 and cb == 3))
                    if first_mm:
                        mm._wait_ge(sM, chunk_dma_count[0])
                        first_mm = False
                    elif t == 1 and wst_wait:
                        mm._wait_ge(sM, wst_cnt)
                        wst_wait = False
                    inc(mm, "P", sP)
                if t % CH == CH - 1 or t == t_steps:
                    chunk_last_reader[ci % 2] = cnt["P"]
                # state matmuls
                if t > 0:
                    for cb in range(4):
                        mm = nc.tensor.matmul(
                            ps[:, cb * bg:(cb + 1) * bg],
                            wst.ap()[:, cb * 128:(cb + 1) * 128],
                            st[:, :], start=False, stop=cb == 3)
                        if cb == 0:
                            mm._wait_ge(sD, ht_cnt[g])
                        inc(mm, "P", sP)
                mm_cnt[g] = cnt["P"]
            for g in range(G):
                bg = BGS[g]
                psv = pss[g][t % PSR].ap()[:, :].rearrange(
                    "p (s n) -> p s n", s=2)
                act = nc.scalar.activation(gates_out_ap(g), psv, Tanh)
                act._wait_ge(sP, mm_cnt[g])
                g_cnt[g] = inc(act, "A", sA)
            def emit_cell(g):
                bg = BGS[g]
                t1 = t1s[g].ap()
                r3 = lambda a: a.rearrange("p (s n) -> p s n", s=2)
                # paired STT: [w|u] = (in+1)*other for pairs (tf,c2x),(ti,tg)
                p1 = nc.vector.scalar_tensor_tensor(
                    r3(uws[g].ap()[:, :]), r3(t1[:, 0:2 * bg]), 1.0,
                    r3(t1[:, 2 * bg:4 * bg]), ADD, MULT)
                p1._wait_ge(sA, g_cnt[g])
                p1_cnt = inc(p1, "D", sD)
                # c2x = 0.5*w + u, written into the t1e c2x block
                cc = nc.vector.scalar_tensor_tensor(
                    t1[:, 2 * bg:3 * bg], uws[g].ap()[:, 0:bg], 0.5,
                    uws[g].ap()[:, bg:2 * bg], MULT, ADD)
                cc._wait_ge(sD, p1_cnt)
                c_cnt[g] = inc(cc, "D", sD)
                ta = nc.scalar.activation(tcs[g].ap()[:, :],
                                          t1[:, 2 * bg:3 * bg],
                                          Tanh, scale=0.5)
                ta._wait_ge(sD, c_cnt[g])
                t_cnt[g] = inc(ta, "A", sA)

            def emit_ht(g):
                bg = BGS[g]
                ht = nc.vector.scalar_tensor_tensor(
                    sts[g][:, :], t1s[g].ap()[:, 4 * bg:5 * bg], 1.0,
                    tcs[g].ap()[:, :], ADD, MULT)
                ht._wait_ge(sA, t_cnt[g])
                ht_cnt[g] = inc(ht, "D", sD)

            for g in range(G):
                emit_cell(g)
            for g in range(G):
                emit_ht(g)

            if t == 0:
                # wipe layer-2 pollution from the bogus step -1
                for g in range(G):
                    bg = BGS[g]
                    m = st_all.ap()[64:128, BOFF[g]:BOFF[g] + BGS[g]]
                    inc(nc.vector.memset(m, 0.0), "D", sD)
                    inc(nc.vector.memset(
                        t1s[g].ap()[64:128, 2 * bg:3 * bg], 0.0), "D", sD)
                    ht_cnt[g] = cnt["D"]

        # output: 2*h2 = st rows 64:128, f16 (host halves and casts)
        od = nc.sync.dma_start(out_d.ap()[:, :], st_all.ap()[64:128, :])
        od._wait_ge(sD, cnt["D"])
        od.then_inc(sM, 16)
        cnt["M"] += 16
        nc.sync.wait_ge(sM, cnt["M"])
        nc.all_engine_barrier()

    nc.compile()
    return nc


def _get_program(t_steps=T):
    key = ("prog", t_steps)
    if key not in _CACHE:
        _CACHE[key] = build_program(t_steps)
    return _CACHE[key]


def make_in_map(inputs, core=0):
    x = np.asarray(inputs["x"], np.float32)
    t_steps = x.shape[1]
    wst, wx = _prep_weights(
        *(np.asarray(inputs[k], np.float32) for k in
          ("W_ih0", "W_hh0", "b_ih0", "b_hh0", "W_ih1", "W_hh1", "b_ih1",
           "b_hh1"))
    )
    xc = x[core * BL:(core + 1) * BL]
    xt = np.ones((4, 512 + t_steps * BL), np.float16)
    xt[:, 0:512] = wx
    xt[0:3, 512:] = xc.transpose(2, 1, 0).reshape(3, t_steps * BL).astype(
        np.float16)
    return {"xt": xt, "wst": wst}


def extract_out(out_mem):
    return out_mem.view(np.float16).reshape(64, BL).T.astype(np.float32) * 0.5


def kernel(x, W_ih0, W_hh0, b_ih0, b_hh0, W_ih1, W_hh1, b_ih1, b_hh1):
    from concourse import bass_utils

    inputs = dict(x=x, W_ih0=W_ih0, W_hh0=W_hh0, b_ih0=b_ih0, b_hh0=b_hh0,
                  W_ih1=W_ih1, W_hh1=W_hh1, b_ih1=b_ih1, b_hh1=b_hh1)
    nc = _get_program(T)
    in_maps = [make_in_map(inputs, core=c) for c in range(NCORES)]
    res = bass_utils.run_bass_kernel_spmd(nc, in_maps, core_ids=list(range(NCORES)))
    outs = [np.asarray(res.results[c]["out"]).T.astype(np.float32) * 0.5
            for c in range(NCORES)]
    return np.concatenate(outs, axis=0).astype(np.float32)


if __name__ == "__main__":
    rng = np.random.default_rng(0)
    s = 1.0 / np.sqrt(H)
    inputs = {
        "x": rng.standard_normal((B, T, I), np.float32),
        "W_ih0": rng.uniform(-s, s, (4 * H, I)).astype(np.float32),
        "W_hh0": rng.uniform(-s, s, (4 * H, H)).astype(np.float32),
        "b_ih0": rng.uniform(-s, s, 4 * H).astype(np.float32),
        "b_hh0": rng.uniform(-s, s, 4 * H).astype(np.float32),
        "W_ih1": rng.uniform(-s, s, (4 * H, H)).astype(np.float32),
        "W_hh1": rng.uniform(-s, s, (4 * H, H)).astype(np.float32),
        "b_ih1": rng.uniform(-s, s, 4 * H).astype(np.float32),
        "b_hh1": rng.uniform(-s, s, 4 * H).astype(np.float32),
    }
    out = kernel(**inputs)
    print(out.shape, out.dtype, np.abs(out).max())
